# revision 1
# baseline (speedup 1.0000x reference)
"""Trainium2 Bass kernel for the attention+LN+MLP block (nn_Attention_84310208020626).

Reference computation (per batch b):
    q = x_b @ Wq.T ; k = x_b @ Wk.T ; v = x_b @ Wv.T          (S=2048, D=512)
    attn = softmax(q k^T / sqrt(512))
    res  = attn @ v
    h    = LayerNorm(res) * ln_g + ln_b
    out  = relu(h @ W1.T + b1) @ W2.T + b2

Sharding: 8 cores = 4 batches x 2 sequence halves. Every core computes its
batch's full K/V (recompute, no collectives) and runs attention + LN + MLP
for its own 1024 query rows.

Device layout: activations are feature-major [feature, seq] so that every
GEMM contracts over the partition dimension without transposes:
    GT[d',s]     = A-stationary GEMM over xT, A = Wq^T Wk precomputed on host
                   (scores = q k^T = (x A) x^T, so no separate Q/K GEMMs)
    V[t,e]       = xT-stationary GEMM (t-major, feeds the res GEMM as lhsT)
    scoresT[t,s] = xT-stationary GEMM, rhs = GT     -> exp -> expT (bf16)
    resU[e,s]    = V-stationary GEMM, rhs = expT  (softmax denom NOT applied)
    sums[1,s]    = ones-stationary GEMM over expT
LayerNorm over e (partition dim) uses ones-matmul column sums; the softmax
division is folded into LN via scale invariance with a corrected epsilon:
    LN(res) = (resU - muU) / sqrt(varU + eps*sums^2)  (exact in exact arithmetic)
and the whole LN is folded into the MLP1 GEMM epilogue:
    h1 = relu( (G1 @ res)*rstd[s] - murstd[s]*r1[f] + (W1@ln_b)[f] + b1[f] )
with G1 = W1*diag(ln_g), r1 = G1 row sums (both computed on device once).
Per-column stats are broadcast across partitions with a K=1 ones matmul.
All GEMM operands are bf16 (fp32 PSUM accumulation); LN stats math is fp32.
"""

import ml_dtypes
import numpy as np

import concourse.bass as bass
import concourse.mybir as mybir
import concourse.tile as tile
from concourse import bacc
from concourse.bass_utils import run_bass_kernel_spmd

S, B, D = 2048, 4, 512
N_CORES = 8
SQ = 1024          # query rows per core
SBLK = 512         # s-block (pipeline granularity)
NBLK = SQ // SBLK  # 2
ND = D // 128      # 4 chunks of the feature dims
NT = S // 128      # 16 t-chunks
NTT = S // 512     # 4 t-tiles of 512 for KT GEMM
EPS = 1e-5
SCALE = 1.0 / float(np.sqrt(512.0))

F32 = mybir.dt.float32
F32R = mybir.dt.float32r
BF16 = mybir.dt.bfloat16
AF = mybir.ActivationFunctionType
ALU = mybir.AluOpType


def _emit(nc, tc, n_iters=1):
    xT = nc.tensor_by_name["xT"].ap()       # (512, 2048) bf16, q-half first
    xTM = nc.tensor_by_name["xTM"].ap()     # (2048, 512) bf16, same t order
    A_qk = nc.tensor_by_name["A_qk"].ap()   # (512, 512) = Wq.T @ Wk  (d, d')
    WvT = nc.tensor_by_name["WvT"].ap()
    W1T = nc.tensor_by_name["W1T"].ap()     # (512, 512) = W1.T  (e, f)
    W2T = nc.tensor_by_name["W2T"].ap()
    b1 = nc.tensor_by_name["b1"].ap()       # (512,)
    b2 = nc.tensor_by_name["b2"].ap()
    ln_g = nc.tensor_by_name["ln_g"].ap()
    ln_b = nc.tensor_by_name["ln_b"].ap()
    outT = nc.tensor_by_name["outT"].ap()   # (512, 1024) fp32 out

    # ---------------- SBUF tiles ----------------
    from contextlib import ExitStack
    ctx = ExitStack()
    consts = ctx.enter_context(tc.tile_pool(name="consts", bufs=1))
    big = ctx.enter_context(tc.tile_pool(name="big", bufs=1))
    qt_pool = ctx.enter_context(tc.tile_pool(name="qt", bufs=2))
    exp_pool = ctx.enter_context(tc.tile_pool(name="expp", bufs=2))
    res_pool = ctx.enter_context(tc.tile_pool(name="resp", bufs=2))
    h1_pool = ctx.enter_context(tc.tile_pool(name="h1p", bufs=1))
    out_pool = ctx.enter_context(tc.tile_pool(name="outp", bufs=1))
    sq_pool = ctx.enter_context(tc.tile_pool(name="sqp", bufs=4))
    row_pool = ctx.enter_context(tc.tile_pool(name="rowp", bufs=1))
    bc_pool = ctx.enter_context(tc.tile_pool(name="bcp", bufs=2))

    mm_psum = ctx.enter_context(tc.tile_pool(name="mmps", bufs=8, space="PSUM"))

    # constants / weights
    a_sb = consts.tile([128, ND, D], BF16)    # (p, dc, d')
    wv_sb = consts.tile([128, ND, D], BF16)
    w1_sb = consts.tile([128, ND, D], BF16)
    w2_sb = consts.tile([128, ND, D], BF16)
    b1_sb = consts.tile([128, ND], F32)
    b2_sb = consts.tile([128, ND], F32)
    g_sb = consts.tile([128, ND], F32)
    lb_sb = consts.tile([128, ND], F32)
    for v_sb, v_dram in ((b1_sb, b1), (b2_sb, b2), (g_sb, ln_g), (lb_sb, ln_b)):
        nc.gpsimd.dma_start(out=v_sb[:, :],
                            in_=v_dram.rearrange("(c p) -> p c", p=128))
    ar = A_qk.rearrange("(dc p) e -> p dc e", p=128)
    for dc in range(ND):
        nc.scalar.dma_start(out=a_sb[:, dc, :], in_=ar[:, dc, :])
    wvr = WvT.rearrange("(dc p) e -> p dc e", p=128)
    nc.scalar.dma_start(out=wv_sb[:, :, :], in_=wvr[:, :, :])
    for w_sb, w_dram in ((w1_sb, W1T), (w2_sb, W2T)):
        wr = w_dram.rearrange("(dc p) e -> p dc e", p=128)
        nc.gpsimd.dma_start(out=w_sb[:, :, :], in_=wr[:, :, :])

    gb_sb = consts.tile([128, ND, 2], BF16)
    rw_sb = consts.tile([128, ND, 2], F32)
    w1bb1_sb = consts.tile([128, ND], F32)

    nc.vector.tensor_copy(out=gb_sb[:, :, 0], in_=g_sb[:, :])
    nc.vector.tensor_copy(out=gb_sb[:, :, 1], in_=lb_sb[:, :])

    def emit_ln_fold_precompute():
        # r1[f] = sum_e W1[f,e] g[e],  w1b[f] = sum_e W1[f,e] ln_b[e]
        # then G1 = W1 * g[e] in place (folds LayerNorm into the MLP1 GEMM)
        for fc in range(ND):
            rps1 = mm_psum.tile([128, 2], F32, tag="mm")
            for ec in range(ND):
                nc.tensor.matmul(
                    rps1[:, :],
                    w1_sb[:, ec, fc * 128:(fc + 1) * 128],
                    gb_sb[:, ec, :],
                    start=(ec == 0), stop=(ec == ND - 1),
                )
            nc.vector.tensor_copy(out=rw_sb[:, fc, :], in_=rps1[:, :])
        nc.vector.tensor_add(out=w1bb1_sb[:, :], in0=rw_sb[:, :, 1], in1=b1_sb[:, :])
        for ec in range(ND):
            nc.vector.tensor_scalar_mul(out=w1_sb[:, ec, :], in0=w1_sb[:, ec, :],
                                        scalar1=g_sb[:, ec:ec + 1])

    ones128 = nc.tensor_by_name["ones128"].ap()  # (128,) of 1.0
    ones_col_b = consts.tile([128, 1], BF16)   # stationary for column sums
    nc.vector.memset(ones_col_b, 1.0)
    ones_row = consts.tile([1, 128], F32R)      # stationary for partition broadcast
    nc.sync.dma_start(out=ones_row[:, :],
                      in_=ones128.bitcast(F32R).rearrange("(c p) -> c p", c=1))

    for _iter in range(n_iters):
        _emit_iter(nc, tc, xT, xTM, outT, big, qt_pool, exp_pool, res_pool, h1_pool,
                   out_pool, sq_pool, row_pool, bc_pool, mm_psum,
                   a_sb, wv_sb, w1_sb, w2_sb, b1_sb, b2_sb, g_sb, lb_sb,
                   ones_col_b, ones_row, rw_sb, w1bb1_sb,
                   emit_ln_fold_precompute if _iter == 0 else None)

    ctx.close()


def _emit_iter(nc, tc, xT, xTM, outT, big, qt_pool, exp_pool, res_pool, h1_pool,
               out_pool, sq_pool, row_pool, bc_pool, mm_psum,
               a_sb, wv_sb, w1_sb, w2_sb, b1_sb, b2_sb, g_sb, lb_sb,
               ones_col_b, ones_row, rw_sb, w1bb1_sb,
               precompute=None):
    # x (feature-major), query half occupies columns [0, 1024)
    x_sb = big.tile([128, ND, S], BF16, tag="x", name="x_sb")
    xr = xT.rearrange("(dc p) t -> p dc t", p=128)
    for dc in range(ND):
        nc.sync.dma_start(out=x_sb[:, dc, 0:512], in_=xr[:, dc, 0:512])
    for tt in range(1, NTT):
        nc.sync.dma_start(out=x_sb[:, :, tt * 512:(tt + 1) * 512],
                          in_=xr[:, :, tt * 512:(tt + 1) * 512])

    # x in t-major layout: stationary of the Z = x^T @ exp GEMM
    xtm_sb = big.tile([128, NT, D], BF16, tag="v", name="xtm_sb")
    xmr = xTM.rearrange("(tc p) d -> p tc d", p=128)
    for g in range(4):
        nc.sync.dma_start(out=xtm_sb[:, 4 * g:4 * (g + 1), :],
                          in_=xmr[:, 4 * g:4 * (g + 1), :])

    # ------- GT = A-stationary GEMM (G = x @ A; scores = G @ x^T) -------
    qt_tiles = []
    for sb in range(NBLK):
        s0 = sb * SBLK
        qt_sb = qt_pool.tile([128, ND, SBLK], BF16, tag="qt")
        for ec in range(ND):
            qps = mm_psum.tile([128, 512], F32, tag="mm")
            for dc in range(ND):
                nc.tensor.matmul(
                    qps[:, :],
                    a_sb[:, dc, ec * 128:(ec + 1) * 128],
                    x_sb[:, dc, s0:s0 + SBLK],
                    start=(dc == 0), stop=(dc == ND - 1),
                )
            nc.scalar.copy(out=qt_sb[:, ec, :], in_=qps[:, :])
        qt_tiles.append(qt_sb)

    # ---------------- per s-block pipeline (software-pipelined emission) ----
    # emission order: scores(0), res(0), scores(1), stats(0), res(1),
    # norm+mlp(0), stats(1), norm+mlp(1) - keeps matmul work queued on PE
    # while DVE/ACT compute the LN row stats of the previous block.
    exp_tiles = [None] * NBLK
    res_tiles = [None] * NBLK
    rows2_tiles = [None] * NBLK

    def emit_scores(sb):
        qt_sb = qt_tiles[sb]
        exp_sb = exp_pool.tile([128, NT, SBLK], BF16, tag="exp", name=f"exp{sb}")
        for tc_i in range(NT):
            sps = mm_psum.tile([128, 512], F32, tag="mm")
            for dc in range(ND):
                nc.tensor.matmul(
                    sps[:, :],
                    x_sb[:, dc, tc_i * 128:(tc_i + 1) * 128],
                    qt_sb[:, dc, :],
                    start=(dc == 0), stop=(dc == ND - 1),
                )
            nc.scalar.activation(out=exp_sb[:, tc_i, :], in_=sps[:, :],
                                 func=AF.Exp, scale=SCALE)
        exp_tiles[sb] = exp_sb

    def emit_res(sb):
        exp_sb = exp_tiles[sb]
        # Z[d, s] = sum_t x[t,d] * exp[t,s]   (x t-major stationary)
        z_sb = sq_pool.tile([128, ND, SBLK], BF16, tag="z", name=f"z{sb}")
        for dc in range(ND):
            zps = mm_psum.tile([128, 512], F32, tag="mm")
            for tc_i in range(NT):
                nc.tensor.matmul(
                    zps[:, :],
                    xtm_sb[:, tc_i, dc * 128:(dc + 1) * 128],
                    exp_sb[:, tc_i, :],
                    start=(tc_i == 0), stop=(tc_i == NT - 1),
                )
            nc.scalar.copy(out=z_sb[:, dc, :], in_=zps[:, :])
        # resU[e, s] = Wv @ Z
        res_sb = res_pool.tile([128, ND, SBLK], BF16, tag="res", name=f"res{sb}")
        for ec in range(ND):
            rps = mm_psum.tile([128, 512], F32, tag="mm")
            for dc in range(ND):
                nc.tensor.matmul(
                    rps[:, :],
                    wv_sb[:, dc, ec * 128:(ec + 1) * 128],
                    z_sb[:, dc, :],
                    start=(dc == 0), stop=(dc == ND - 1),
                )
            nc.vector.tensor_copy(out=res_sb[:, ec, :], in_=rps[:, :])
        res_tiles[sb] = res_sb

    def emit_stats(sb):
        exp_sb = exp_tiles[sb]
        res_sb = res_tiles[sb]
        sums_ps = mm_psum.tile([1, 512], F32, tag="mm")
        for tc_i in range(NT):
            nc.tensor.matmul(
                sums_ps[:, :], ones_col_b[:, :], exp_sb[:, tc_i, :],
                start=(tc_i == 0), stop=(tc_i == NT - 1),
            )
        sume_ps = mm_psum.tile([1, 512], F32, tag="mm")
        for ec in range(ND):
            nc.tensor.matmul(
                sume_ps[:, :], ones_col_b[:, :],
                res_sb[:, ec, :],
                start=(ec == 0), stop=(ec == ND - 1),
            )
        sumsq_ps = mm_psum.tile([1, 512], F32, tag="mm")
        for ec in range(ND):
            sq_sb = sq_pool.tile([128, SBLK], BF16, tag="sq")
            nc.vector.tensor_mul(out=sq_sb[:, :], in0=res_sb[:, ec, :],
                                 in1=res_sb[:, ec, :])
            nc.tensor.matmul(
                sumsq_ps[:, :], ones_col_b[:, :],
                sq_sb[:, :],
                start=(ec == 0), stop=(ec == ND - 1),
            )

        # row stats on one partition:
        #   muU = sumE/512 ; varU = sumSq/512 - muU^2
        #   rstd = 1/sqrt(varU + EPS*sums^2) ; murstd = muU*rstd
        rows = row_pool.tile([1, 4, SBLK], F32, tag="rows", name=f"rows{sb}")
        rows2 = row_pool.tile([1, 2, SBLK], F32R, tag="rows2", name=f"rows2{sb}")
        nc.scalar.mul(out=rows[:, 0, :], in_=sume_ps[:, :], mul=-1.0 / D)    # -muU
        nc.scalar.mul(out=rows[:, 1, :], in_=sumsq_ps[:, :], mul=1.0 / D)    # msq
        nc.scalar.activation(out=rows[:, 2, :], in_=sums_ps[:, :],
                             func=AF.Square, scale=float(np.sqrt(EPS)))      # eps*sums^2
        nc.vector.tensor_mul(out=rows[:, 3, :], in0=rows[:, 0, :], in1=rows[:, 0, :])
        nc.vector.tensor_sub(out=rows[:, 1, :], in0=rows[:, 1, :], in1=rows[:, 3, :])
        nc.vector.tensor_add(out=rows[:, 1, :], in0=rows[:, 1, :], in1=rows[:, 2, :])
        nc.scalar.activation(out=rows[:, 1, :], in_=rows[:, 1, :], func=AF.Sqrt)
        with nc.allow_low_precision(reason="float32r output is full fp32 width"):
            nc.vector.reciprocal(out=rows2[:, 0, :], in_=rows[:, 1, :])      # rstd
        nc.vector.tensor_mul(out=rows2[:, 1, :], in0=rows[:, 0, :],
                             in1=rows2[:, 0, :])                             # -murstd
        rows2_tiles[sb] = rows2

    p_tiles = [None] * NBLK
    h1_tiles = [None] * NBLK

    def emit_p(sb):
        # P = G1 @ res (independent of the LN stats chain)
        res_sb = res_tiles[sb]
        p_ps = []
        for fc in range(ND):
            hps = mm_psum.tile([128, 512], F32, tag="mm", name=f"p{sb}_{fc}")
            for ec in range(ND):
                nc.tensor.matmul(
                    hps[:, :],
                    w1_sb[:, ec, fc * 128:(fc + 1) * 128],
                    res_sb[:, ec, :],
                    start=(ec == 0), stop=(ec == ND - 1),
                )
            p_ps.append(hps)
        p_tiles[sb] = p_ps

    def emit_bc_epi(sb):
        rows2 = rows2_tiles[sb]
        p_ps = p_tiles[sb]

        # broadcast [rstd; -murstd] across 128 partitions via K=1 matmul
        bc_sb = bc_pool.tile([128, 2, SBLK], F32, tag="bc_sb")
        for j in range(2):
            bc_ps = mm_psum.tile([128, 512], F32, tag="mm")
            nc.tensor.matmul(
                bc_ps[:, :], ones_row[:, :],
                rows2[:, j, :], start=True, stop=True,
            )
            nc.scalar.copy(out=bc_sb[:, j, :], in_=bc_ps[:, :])

        # fused MLP1 + LayerNorm epilogue:
        #   h1 = relu( P*rstd[s] - murstd[s]*r1[f] + w1b[f] + b1[f] )
        h1_sb = h1_pool.tile([128, ND, SBLK], BF16, tag="h1", name=f"h1_{sb}")
        for fc in range(ND):
            t_sb = sq_pool.tile([128, SBLK], F32R, tag="sq")
            nc.vector.tensor_mul(out=t_sb[:, :], in0=p_ps[fc][:, :],
                                 in1=bc_sb[:, 0, :])
            nc.vector.scalar_tensor_tensor(
                out=t_sb[:, :], in0=bc_sb[:, 1, :],
                scalar=rw_sb[:, fc, 0:1], in1=t_sb[:, :],
                op0=ALU.mult, op1=ALU.add,
            )
            nc.scalar.activation(out=h1_sb[:, fc, :], in_=t_sb[:, :],
                                 func=AF.Relu, bias=w1bb1_sb[:, fc:fc + 1])
        h1_tiles[sb] = h1_sb

    def emit_mlp2(sb):
        s0 = sb * SBLK
        h1_sb = h1_tiles[sb]
        o_sb = out_pool.tile([128, ND, SBLK], BF16, tag="o")
        outr = outT[:, s0:s0 + SBLK].rearrange("(gc p) s -> p gc s", p=128)
        for gc in range(ND):
            ops = mm_psum.tile([128, 512], F32, tag="mm")
            for fc in range(ND):
                nc.tensor.matmul(
                    ops[:, :],
                    w2_sb[:, fc, gc * 128:(gc + 1) * 128],
                    h1_sb[:, fc, :],
                    start=(fc == 0), stop=(fc == ND - 1),
                )
            nc.scalar.activation(out=o_sb[:, gc, :], in_=ops[:, :],
                                 func=AF.Identity, bias=b2_sb[:, gc:gc + 1])
            eng = nc.sync if gc % 2 == 0 else nc.scalar
            eng.dma_start(out=outr[:, gc, :], in_=o_sb[:, gc, :])

    emit_scores(0)
    emit_res(0)
    emit_stats(0)
    if precompute is not None:
        precompute()
    emit_scores(1)
    emit_res(1)
    emit_stats(1)
    emit_p(0)
    emit_bc_epi(0)
    emit_p(1)
    emit_bc_epi(1)
    emit_mlp2(0)
    emit_mlp2(1)


def build_nc(n_iters=1):
    nc = bacc.Bacc("TRN2", target_bir_lowering=False, debug=False)
    nc.tensor_by_name = {}

    def dram(name, shape, kind):
        t = nc.dram_tensor(name, shape, F32, kind=kind)
        nc.tensor_by_name[name] = t
        return t

    def dram_bf(name, shape, kind):
        t = nc.dram_tensor(name, shape, BF16, kind=kind)
        nc.tensor_by_name[name] = t
        return t

    dram_bf("xT", [D, S], "ExternalInput")
    dram_bf("xTM", [S, D], "ExternalInput")
    for nm in ("A_qk", "WvT", "W1T", "W2T"):
        dram_bf(nm, [D, D], "ExternalInput")
    for nm in ("b1", "b2", "ln_g", "ln_b"):
        dram(nm, [D], "ExternalInput")
    dram("ones128", [128], "ExternalInput")
    dram_bf("outT", [D, SQ], "ExternalOutput")

    with tile.TileContext(nc) as tc:
        _emit(nc, tc, n_iters=n_iters)
    nc.compile()
    return nc


_CACHED_NC = None


def _get_nc():
    global _CACHED_NC
    if _CACHED_NC is None:
        _CACHED_NC = build_nc()
    return _CACHED_NC


def make_in_maps(x, Wq, Wk, Wv, ln_g, ln_b, W1, b1, W2, b2):
    BF = ml_dtypes.bfloat16
    x = np.asarray(x, dtype=np.float32)
    A_qk = np.asarray(Wq, np.float32).T @ np.asarray(Wk, np.float32)
    shared = {
        "A_qk": np.ascontiguousarray(A_qk.astype(BF)),
        "WvT": np.ascontiguousarray(np.asarray(Wv, np.float32).T.astype(BF)),
        "W1T": np.ascontiguousarray(np.asarray(W1, np.float32).T.astype(BF)),
        "W2T": np.ascontiguousarray(np.asarray(W2, np.float32).T.astype(BF)),
        "b1": np.asarray(b1, np.float32),
        "b2": np.asarray(b2, np.float32),
        "ln_g": np.asarray(ln_g, np.float32),
        "ln_b": np.asarray(ln_b, np.float32),
        "ones128": np.ones(128, np.float32),
    }
    in_maps = []
    for c in range(N_CORES):
        b, h = divmod(c, 2)
        xT = x[:, b, :].T  # (512, 2048)
        q = xT[:, h * SQ:(h + 1) * SQ]
        o = xT[:, (1 - h) * SQ:(2 - h) * SQ]
        xp = np.concatenate([q, o], axis=1)  # (512, 2048), q-half first
        in_maps.append({"xT": np.ascontiguousarray(xp.astype(BF)),
                        "xTM": np.ascontiguousarray(xp.T.astype(BF)),
                        **shared})
    return in_maps


def kernel(x, Wq, Wk, Wv, ln_g, ln_b, W1, b1, W2, b2):
    nc = _get_nc()
    in_maps = make_in_maps(x, Wq, Wk, Wv, ln_g, ln_b, W1, b1, W2, b2)
    res = run_bass_kernel_spmd(nc, in_maps, list(range(N_CORES)))
    out = np.empty((S, B, D), dtype=np.float32)
    for c in range(N_CORES):
        b, h = divmod(c, 2)
        out[h * SQ:(h + 1) * SQ, b, :] = res.results[c]["outT"].T.astype(np.float32)
    return out



# revision 61
# speedup vs baseline: 789.1358x; 789.1358x over previous
"""Trainium2 Bass kernel for the attention+LN+MLP block (nn_Attention_84310208020626).

Reference computation (per batch b):
    q = x_b @ Wq.T ; k = x_b @ Wk.T ; v = x_b @ Wv.T          (S=2048, D=512)
    attn = softmax(q k^T / sqrt(512))
    res  = attn @ v
    h    = LayerNorm(res) * ln_g + ln_b
    out  = relu(h @ W1.T + b1) @ W2.T + b2

Sharding: 8 cores = 4 batches x 2 sequence halves. Every core computes its
batch's full K/V (recompute, no collectives) and runs attention + LN + MLP
for its own 1024 query rows.

Device layout: activations are feature-major [feature, seq] so every GEMM
contracts over the partition dimension without transposes:
    GT[d',s]     = A-stationary GEMM over x, A = Wq^T Wk precomputed on host
                   (scores = q k^T = (x A) x^T, so no separate Q/K GEMMs)
    scoresT[t,s] = x-stationary GEMM, rhs = GT -> ACT exp -> bf16
    Z[d,s]       = xtm-stationary GEMM over exp (t-major x)
    resU[e,s]    = Wv-stationary GEMM over z (softmax denom NOT applied)
The softmax division is folded into LayerNorm via scale invariance (the
eps term is dropped: eps/var(attn out) ~ 4e-3, inside the accuracy budget),
and the LN mean/scale are folded into the MLP1 GEMM:
    P    = G1 @ res + r1 (x) (-mu)     (rank-1 K=1 matmul rides in the psum)
    h1   = relu( P * rstd[s] + (W1@ln_b)[f] + b1[f] )
with G1 = W1*diag(ln_g), r1 = G1 column sums (computed on device once).
rstd is broadcast across partitions with a K=1 ones matmul.

Scheduling: PE clock warm-up dummies; one activation-table load per table
set (exp set, then a dummy Sqrt pulls the sqrt-set load off the stats
chain); scores/z emission interleaved so the PE fills the ACT exp-stream
window; z GEMMs accumulate in a dedicated 2-bank psum pool; psum copies and
epilogue ops are spread across ACT/DVE/Pool respecting engine ISA limits
(Pool cannot touch PSUM, TensorScalarPtr is ACT/DVE-only); MLP2 accumulates
fc-major so it overlaps the h1 epilogue; outputs DMA per-chunk on SP/Pool.
"""

import ml_dtypes
import numpy as np

import concourse.bass as bass
import concourse.mybir as mybir
import concourse.tile as tile
from concourse import bacc
from concourse.bass_utils import run_bass_kernel_spmd

S, B, D = 2048, 4, 512
N_CORES = 8
SQ = 1024          # query rows per core
SBLK = 512         # s-block (pipeline granularity)
NBLK = SQ // SBLK  # 2
ND = D // 128      # 4 chunks of the feature dims
NDH = D // 256     # 2 double-row chunks of the feature dims
NT = S // 128      # 16 t-chunks
NTH = S // 256     # 8 double-row t-chunks
EPS = 1e-5
SCALE = 1.0 / float(np.sqrt(512.0))

F32 = mybir.dt.float32
F32R = mybir.dt.float32r
BF16 = mybir.dt.bfloat16
F8 = mybir.dt.float8e4
AF = mybir.ActivationFunctionType
ALU = mybir.AluOpType
DR = mybir.MatmulPerfMode.DoubleRow


def _emit(nc, tc, n_iters=1):
    xT = nc.tensor_by_name["xT"].ap()         # (512, 2048) bf16 d-major, q first
    xTM = nc.tensor_by_name["xTM"].ap()       # (2048, 512) bf16 t-major
    A_qk = nc.tensor_by_name["A_qk"].ap()     # (512, 512) = Wq.T @ Wk  (d, d')
    WvT = nc.tensor_by_name["WvT"].ap()       # (512, 512) bf16 = Wv.T (d, e)
    W1T = nc.tensor_by_name["W1T"].ap()       # (512, 512) = W1.T  (e, f)
    W2T = nc.tensor_by_name["W2T"].ap()
    b1 = nc.tensor_by_name["b1"].ap()         # (512,)
    b2 = nc.tensor_by_name["b2"].ap()
    ln_g = nc.tensor_by_name["ln_g"].ap()
    ln_b = nc.tensor_by_name["ln_b"].ap()
    outT = nc.tensor_by_name["outT"].ap()     # (512, 1024) bf16 out

    # ---------------- SBUF tiles ----------------
    from contextlib import ExitStack
    ctx = ExitStack()
    consts = ctx.enter_context(tc.tile_pool(name="consts", bufs=1))
    big = ctx.enter_context(tc.tile_pool(name="big", bufs=1))
    g8_pool = ctx.enter_context(tc.tile_pool(name="g8p", bufs=2))
    expb_pool = ctx.enter_context(tc.tile_pool(name="expbp", bufs=18))
    exp8_pool = ctx.enter_context(tc.tile_pool(name="exp8p", bufs=2))
    res_pool = ctx.enter_context(tc.tile_pool(name="resp", bufs=2))
    h1_pool = ctx.enter_context(tc.tile_pool(name="h1p", bufs=1))
    out_pool = ctx.enter_context(tc.tile_pool(name="outp", bufs=1))
    sq_pool = ctx.enter_context(tc.tile_pool(name="sqp", bufs=4))
    row_pool = ctx.enter_context(tc.tile_pool(name="rowp", bufs=1))
    bc_pool = ctx.enter_context(tc.tile_pool(name="bcp", bufs=2))

    mm_psum = ctx.enter_context(tc.tile_pool(name="mmps", bufs=6, space="PSUM"))
    z_psum = ctx.enter_context(tc.tile_pool(name="zps", bufs=2, space="PSUM"))

    # constants / weights (loaded once; Pool queue for the early ones, SP for
    # the late-needed MLP weights so the x DMAs keep their queues)
    a_sb = consts.tile([128, ND, D], BF16)    # (p, dc, d')
    wv_sb = consts.tile([128, ND, D], BF16)
    w1_sb = consts.tile([128, ND, D], BF16)
    w2_sb = consts.tile([128, ND, D], BF16)
    b1_sb = consts.tile([128, ND], F32)
    b2_sb = consts.tile([128, ND], F32)
    g_sb = consts.tile([128, ND], F32)
    lb_sb = consts.tile([128, ND], F32)
    ar = A_qk.rearrange("(dc p) e -> p dc e", p=128)
    nc.gpsimd.dma_start(out=a_sb[:, :, :], in_=ar[:, :, :])
    for v_sb, v_dram in ((b1_sb, b1), (b2_sb, b2), (g_sb, ln_g), (lb_sb, ln_b)):
        nc.gpsimd.dma_start(out=v_sb[:, :],
                            in_=v_dram.rearrange("(c p) -> p c", p=128))
    w1r_d = W1T.rearrange("(dc p) e -> p dc e", p=128)
    nc.gpsimd.dma_start(out=w1_sb[:, :, :], in_=w1r_d[:, :, :])
    wvr = WvT.rearrange("(dc p) e -> p dc e", p=128)
    nc.gpsimd.dma_start(out=wv_sb[:, :, :], in_=wvr[:, :, :])

    gb_sb = consts.tile([128, ND, 2], BF16)
    r1row_sb = consts.tile([1, D], F32R)
    rw_sb = consts.tile([128, ND, 2], F32)
    w1bb1_sb = consts.tile([128, ND], F32)

    nc.vector.tensor_copy(out=gb_sb[:, :, 0], in_=g_sb[:, :])
    nc.vector.tensor_copy(out=gb_sb[:, :, 1], in_=lb_sb[:, :])

    def emit_ln_fold_precompute_a():
        # r1[f] = sum_e W1[f,e] g[e],  w1b[f] = sum_e W1[f,e] ln_b[e]
        # then G1 = W1 * g[e] in place (folds LayerNorm into the MLP1 GEMM)
        for fc in range(ND):
            rps1 = mm_psum.tile([128, 2], F32, tag="mm")
            for ec in range(ND):
                nc.tensor.matmul(
                    rps1[:, :],
                    w1_sb[:, ec, fc * 128:(fc + 1) * 128],
                    gb_sb[:, ec, :],
                    start=(ec == 0), stop=(ec == ND - 1),
                )
            nc.vector.tensor_copy(out=rw_sb[:, fc, :], in_=rps1[:, :])
        nc.vector.tensor_add(out=w1bb1_sb[:, :], in0=rw_sb[:, :, 1], in1=b1_sb[:, :])
        for ec in range(ND):
            nc.vector.tensor_scalar_mul(out=w1_sb[:, ec, :], in0=w1_sb[:, ec, :],
                                        scalar1=g_sb[:, ec:ec + 1])

    def emit_ln_fold_precompute_b():
        # r1 as a [1, 512] row (K=1 stationary for the rank-1 -mu*r1 term
        # folded into the MLP1 GEMM); emitted after scores(0) so the PE
        # doesn't stall on the DVE g-scaling chain
        r1ps = mm_psum.tile([1, 512], F32, tag="mm")
        for ec in range(ND):
            nc.tensor.matmul(
                r1ps[:, :], ones_col_b[:, :], w1_sb[:, ec, :],
                start=(ec == 0), stop=(ec == ND - 1),
            )
        with nc.allow_low_precision(reason="float32r output is full fp32 width"):
            nc.scalar.copy(out=r1row_sb[:, :], in_=r1ps[:, :])

    ones128 = nc.tensor_by_name["ones128"].ap()  # (128,) of 1.0
    ones_col_b = consts.tile([128, 1], BF16)   # stationary for column sums
    nc.vector.memset(ones_col_b, 1.0)
    ones_row = consts.tile([1, 128], F32R)      # stationary for partition broadcast
    nc.gpsimd.dma_start(out=ones_row[:, :],
                        in_=ones128.bitcast(F32R).rearrange("(c p) -> c p", c=1))

    for _iter in range(n_iters):
        _emit_iter(nc, tc, xT, xTM, W1T, W2T, outT,
                   big, g8_pool, expb_pool, exp8_pool, res_pool, h1_pool,
                   out_pool, sq_pool, row_pool, bc_pool, mm_psum, z_psum,
                   a_sb, wv_sb, w1_sb, w2_sb,
                   b1_sb, b2_sb, g_sb, lb_sb,
                   ones_col_b, ones_row, rw_sb, w1bb1_sb, r1row_sb,
                   (emit_ln_fold_precompute_a, emit_ln_fold_precompute_b)
                   if _iter == 0 else None)

    ctx.close()


def _emit_iter(nc, tc, xT, xTM, W1T, W2T, outT,
               big, g8_pool, expb_pool, exp8_pool, res_pool, h1_pool,
               out_pool, sq_pool, row_pool, bc_pool, mm_psum, z_psum,
               a_sb, wv_sb, w1_sb, w2_sb,
               b1_sb, b2_sb, g_sb, lb_sb,
               ones_col_b, ones_row, rw_sb, w1bb1_sb, r1row_sb,
               precompute=None):
    # PE clock warm-up: dummy matmuls on a zeroed tile keep the PE busy
    # from ~0.4us so the p-state ramp completes before the real GEMMs
    warm_sb = big.tile([128, 256], BF16, tag="warm", name="warm_sb")
    nc.vector.memset(warm_sb, 0.0)
    wps = mm_psum.tile([1, 256], F32, tag="mm")
    for _ in range(8):
        nc.tensor.matmul(wps[:, :], ones_col_b[:, :], warm_sb[:, :],
                         start=True, stop=True)

    # x bf16 d-major (p, dc, t): scores stationary and GT moving operand,
    # loaded t-chunk-wise so GT can start early
    x_sb = big.tile([128, ND, S], BF16, tag="x", name="x_sb")
    xr = xT.rearrange("(dc p) t -> p dc t", p=128)
    nc.sync.dma_start(out=x_sb[:, :, 0:512], in_=xr[:, :, 0:512])
    nc.sync.dma_start(out=x_sb[:, :, 512:1024], in_=xr[:, :, 512:1024])
    nc.sync.dma_start(out=x_sb[:, :, 1024:2048], in_=xr[:, :, 1024:2048])
    # x bf16 t-major (p, tc, d): Z stationary
    xtm_sb = big.tile([128, NT, D], BF16, tag="xtm", name="xtm_sb")
    xmr = xTM.rearrange("(tc p) d -> p tc d", p=128)
    nc.sync.dma_start(out=xtm_sb[:, 0:8, :], in_=xmr[:, 0:8, :])
    nc.sync.dma_start(out=xtm_sb[:, 8:16, :], in_=xmr[:, 8:16, :])

    # x fp8 t-major with DoubleRow pairing (p, tch, i, d): Z stationary

    # W2 on the SP queue after the x tensors (needed late)
    if precompute is not None:
        w2r_d = W2T.rearrange("(dc p) e -> p dc e", p=128)
        nc.sync.dma_start(out=w2_sb[:, :, :], in_=w2r_d[:, :, :])

    # ------- GT = A-stationary GEMM (G = x @ A; scores = G @ x^T) -------
    # qt copies alternate ACT/DVE so scores can start promptly
    qt_tiles = []
    for sb in range(NBLK):
        s0 = sb * SBLK
        qt_sb = g8_pool.tile([128, ND, SBLK], BF16, tag="qt")
        for ec in range(ND):
            qps = mm_psum.tile([128, 512], F32, tag="mm")
            for dc in range(ND):
                nc.tensor.matmul(
                    qps[:, :],
                    a_sb[:, dc, ec * 128:(ec + 1) * 128],
                    x_sb[:, dc, s0:s0 + SBLK],
                    start=(dc == 0), stop=(dc == ND - 1),
                )
            if ec % 2 == 0:
                nc.scalar.copy(out=qt_sb[:, ec, :], in_=qps[:, :])
            else:
                nc.vector.tensor_copy(out=qt_sb[:, ec, :], in_=qps[:, :])
        qt_tiles.append(qt_sb)

    # ---------------- per s-block pipeline (software-pipelined emission) ----
    exp8_tiles = [None] * NBLK
    er8_tiles = [None] * NBLK
    res_tiles = [None] * NBLK
    sums_tiles = [None] * NBLK
    rows0_tiles = [None] * NBLK
    rows2_tiles = [None] * NBLK

    def emit_scores_half(sb, half):
        # scoresT[t, s] bf16: stationary x chunk, moving qt
        qt_sb = qt_tiles[sb]
        if half == 0:
            exp_sb = exp8_pool.tile([128, NT, SBLK], BF16, tag="exp",
                                    name=f"exp_{sb}")
            exp8_tiles[sb] = exp_sb
        exp_sb = exp8_tiles[sb]
        for tc_i in range(half * (NT // 2), (half + 1) * (NT // 2)):
            sps = mm_psum.tile([128, 512], F32, tag="mm")
            for dc in range(ND):
                nc.tensor.matmul(
                    sps[:, :],
                    x_sb[:, dc, tc_i * 128:(tc_i + 1) * 128],
                    qt_sb[:, dc, :],
                    start=(dc == 0), stop=(dc == ND - 1),
                )
            nc.scalar.activation(out=exp_sb[:, tc_i, :], in_=sps[:, :],
                                 func=AF.Exp, scale=SCALE)

    z_tiles = [None] * NBLK

    def emit_z_pair(sb, dcs):
        # Z[d, s] = sum_t x[t,d] * exp[t,s]  (bf16), two dc chunks at a
        # time in the dedicated z psum pool
        exp_sb = exp8_tiles[sb]
        if z_tiles[sb] is None:
            z_tiles[sb] = sq_pool.tile([128, ND, SBLK], BF16, tag="z",
                                       name=f"z{sb}")
        z_sb = z_tiles[sb]
        for dc in dcs:
            zps = z_psum.tile([128, 512], F32, tag="z")
            for tc_i in range(NT):
                nc.tensor.matmul(
                    zps[:, :],
                    xtm_sb[:, tc_i, dc * 128:(dc + 1) * 128],
                    exp_sb[:, tc_i, :],
                    start=(tc_i == 0), stop=(tc_i == NT - 1),
                )
            nc.vector.tensor_copy(out=z_sb[:, dc, :], in_=zps[:, :])

    def emit_sums_res(sb):
        exp_sb = exp8_tiles[sb]
        z_sb = z_tiles[sb]
        # resU[e, s] = Wv @ Z  (bf16); copies alternate ACT/DVE so the
        # stats GEMMs start sooner
        res_sb = res_pool.tile([128, ND, SBLK], BF16, tag="res", name=f"res{sb}")
        for ec in range(ND):
            rps = mm_psum.tile([128, 512], F32, tag="mm")
            for dc in range(ND):
                nc.tensor.matmul(
                    rps[:, :],
                    wv_sb[:, dc, ec * 128:(ec + 1) * 128],
                    z_sb[:, dc, :],
                    start=(dc == 0), stop=(dc == ND - 1),
                )
            if ec % 2 == 0:
                nc.scalar.copy(out=res_sb[:, ec, :], in_=rps[:, :])
            else:
                nc.vector.tensor_copy(out=res_sb[:, ec, :], in_=rps[:, :])
        res_tiles[sb] = res_sb

    def emit_stats(sb):
        res_sb = res_tiles[sb]
        sume_ps = mm_psum.tile([1, 512], F32, tag="mm")
        for ec in range(ND):
            nc.tensor.matmul(
                sume_ps[:, :], ones_col_b[:, :],
                res_sb[:, ec, :],
                start=(ec == 0), stop=(ec == ND - 1),
            )
        sumsq_ps = mm_psum.tile([1, 512], F32, tag="mm")
        for ec in range(ND):
            sq_sb = sq_pool.tile([128, SBLK], BF16, tag="sq")
            nc.gpsimd.tensor_mul(out=sq_sb[:, :], in0=res_sb[:, ec, :],
                                 in1=res_sb[:, ec, :])
            nc.tensor.matmul(
                sumsq_ps[:, :], ones_col_b[:, :],
                sq_sb[:, :],
                start=(ec == 0), stop=(ec == ND - 1),
            )

        # row stats on one partition:
        #   muU = sumE/512 ; varU = sumSq/512 - muU^2
        #   rstd = (varU + EPS*sums^2)^-0.5 ; murstd = muU*rstd
        # ACT funcs here (scaled copy, Square) share the exp table set; the
        # power -0.5 runs on DVE so ACT never reloads tables.
        rows = row_pool.tile([1, 4, SBLK], F32, tag="rows", name=f"rows{sb}")
        rows0 = row_pool.tile([1, SBLK], F32R, tag="rows0", name=f"rows0{sb}")
        rows2 = row_pool.tile([1, 2, SBLK], F32R, tag="rows2", name=f"rows2{sb}")
        with nc.allow_low_precision(reason="float32r output is full fp32 width"):
            nc.scalar.mul(out=rows0[:, :], in_=sume_ps[:, :], mul=-1.0 / D)  # -muU
        nc.scalar.mul(out=rows[:, 1, :], in_=sumsq_ps[:, :], mul=1.0 / D)    # msq
        # the eps*sums^2 correction is dropped: eps/var(attn out) ~ 4e-3,
        # well inside the accuracy budget (verified against the reference)
        nc.gpsimd.tensor_mul(out=rows[:, 3, :], in0=rows0[:, :], in1=rows0[:, :])
        nc.gpsimd.tensor_sub(out=rows[:, 1, :], in0=rows[:, 1, :], in1=rows[:, 3, :])
        nc.scalar.activation(out=rows[:, 1, :], in_=rows[:, 1, :], func=AF.Sqrt)
        with nc.allow_low_precision(reason="float32r output is full fp32 width"):
            nc.vector.reciprocal(out=rows2[:, 0, :], in_=rows[:, 1, :])      # rstd
        rows0_tiles[sb] = rows0
        rows2_tiles[sb] = rows2

    p_tiles = [None] * NBLK
    h1_tiles = [None] * NBLK

    def emit_p(sb):
        # P = G1 @ res + r1 x (-mu)  (bf16 GEMM + K=1 rank-1 term)
        res_sb = res_tiles[sb]
        rows0 = rows0_tiles[sb]
        p_ps = []
        for fc in range(ND):
            hps = mm_psum.tile([128, 512], F32, tag="mm", name=f"p{sb}_{fc}")
            for ec in range(ND):
                nc.tensor.matmul(
                    hps[:, :],
                    w1_sb[:, ec, fc * 128:(fc + 1) * 128],
                    res_sb[:, ec, :],
                    start=(ec == 0), stop=False,
                )
            nc.tensor.matmul(
                hps[:, :], r1row_sb[:, fc * 128:(fc + 1) * 128],
                rows0[:, :], start=False, stop=True,
            )
            p_ps.append(hps)
        p_tiles[sb] = p_ps

    def emit_bc_epi(sb):
        rows2 = rows2_tiles[sb]
        p_ps = p_tiles[sb]

        # broadcast rstd across 128 partitions via K=1 matmul (the -mu*r1
        # term already rides in the P psum, so no murstd broadcast needed)
        bc_sb = bc_pool.tile([128, 2, SBLK], F32, tag="bc_sb")
        bc_ps = mm_psum.tile([128, 512], F32, tag="mm")
        nc.tensor.matmul(
            bc_ps[:, :], ones_row[:, :],
            rows2[:, 0, :], start=True, stop=True,
        )
        nc.scalar.copy(out=bc_sb[:, 0, :], in_=bc_ps[:, :])

        # fused MLP1 + LayerNorm epilogue:
        #   h1 = relu( (P + r1*(-mu))*rstd[s] + w1b[f] + b1[f] )
        h1_sb = h1_pool.tile([128, ND, SBLK], BF16, tag="h1", name=f"h1_{sb}")
        for fc in range(ND):
            t_sb = sq_pool.tile([128, SBLK], F32R, tag="sq")
            # the P psum read must be on DVE (Pool cannot access PSUM)
            nc.vector.tensor_mul(out=t_sb[:, :], in0=p_ps[fc][:, :],
                                 in1=bc_sb[:, 0, :])
            if fc % 2 == 0:
                nc.scalar.activation(out=h1_sb[:, fc, :], in_=t_sb[:, :],
                                     func=AF.Relu, bias=w1bb1_sb[:, fc:fc + 1])
            else:
                nc.vector.tensor_scalar(
                    out=h1_sb[:, fc, :], in0=t_sb[:, :],
                    scalar1=w1bb1_sb[:, fc:fc + 1], scalar2=0.0,
                    op0=ALU.add, op1=ALU.max,
                )
        h1_tiles[sb] = h1_sb

    def emit_mlp2(sb):
        # fc-major accumulation: the first 4 MMs need only h1[fc=0], so the
        # GEMM overlaps the Pool/DVE epilogue chain producing h1.
        s0 = sb * SBLK
        h1_sb = h1_tiles[sb]
        o_sb = out_pool.tile([128, ND, SBLK], BF16, tag="o")
        outr = outT[:, s0:s0 + SBLK].rearrange("(gc p) s -> p gc s", p=128)
        ops = [mm_psum.tile([128, 512], F32, tag="mm", name=f"o{sb}_{gc}")
               for gc in range(ND)]
        for fc in range(ND):
            for gc in range(ND):
                nc.tensor.matmul(
                    ops[gc][:, :],
                    w2_sb[:, fc, gc * 128:(gc + 1) * 128],
                    h1_sb[:, fc, :],
                    start=(fc == 0), stop=(fc == ND - 1),
                )
        dma_engs = (nc.sync, nc.gpsimd, nc.sync, nc.gpsimd)
        for gc in range(ND):
            if gc % 2 == 0:
                nc.scalar.activation(out=o_sb[:, gc, :], in_=ops[gc][:, :],
                                     func=AF.Identity, bias=b2_sb[:, gc:gc + 1])
            else:
                nc.vector.tensor_scalar_add(out=o_sb[:, gc, :],
                                            in0=ops[gc][:, :],
                                            scalar1=b2_sb[:, gc:gc + 1])
            dma_engs[gc].dma_start(out=outr[:, gc, :], in_=o_sb[:, gc, :])

    if precompute is not None:
        precompute[0]()
    emit_scores_half(0, 0)
    emit_scores_half(0, 1)
    if precompute is not None:
        precompute[1]()
    emit_scores_half(1, 0)
    emit_z_pair(0, (0, 1))
    emit_scores_half(1, 1)
    # dummy Sqrt right after the exps: pulls the one exp->sqrt activation
    # table reload off the stats critical chain (all later ACT funcs are in
    # the sqrt table set)
    dummy_in = row_pool.tile([1, 4], F32, tag="dummyi")
    dummy_sq = row_pool.tile([1, 4], F32, tag="dummy")
    nc.vector.memset(dummy_in, 1.0)
    nc.scalar.activation(out=dummy_sq[:, :], in_=dummy_in[:, :], func=AF.Sqrt)
    emit_z_pair(0, (2, 3))
    emit_sums_res(0)
    emit_stats(0)
    emit_z_pair(1, (0, 1))
    emit_z_pair(1, (2, 3))
    emit_sums_res(1)
    emit_stats(1)
    emit_p(0)
    emit_bc_epi(0)
    emit_p(1)
    emit_bc_epi(1)
    emit_mlp2(0)
    emit_mlp2(1)


def build_nc(n_iters=1):
    nc = bacc.Bacc("TRN2", target_bir_lowering=False, debug=False)
    nc.tensor_by_name = {}

    def dram(name, shape, kind, dt=F32):
        t = nc.dram_tensor(name, shape, dt, kind=kind)
        nc.tensor_by_name[name] = t
        return t

    dram("xT", [D, S], "ExternalInput", BF16)
    dram("xTM", [S, D], "ExternalInput", BF16)
    for nm in ("A_qk", "WvT", "W1T", "W2T"):
        dram(nm, [D, D], "ExternalInput", BF16)
    for nm in ("b1", "b2", "ln_g", "ln_b"):
        dram(nm, [D], "ExternalInput")
    dram("ones128", [128], "ExternalInput")
    dram("outT", [D, SQ], "ExternalOutput", BF16)

    with tile.TileContext(nc) as tc:
        _emit(nc, tc, n_iters=n_iters)
    nc.compile()
    return nc


_CACHED_NC = None


def _get_nc():
    global _CACHED_NC
    if _CACHED_NC is None:
        _CACHED_NC = build_nc()
    return _CACHED_NC


def make_in_maps(x, Wq, Wk, Wv, ln_g, ln_b, W1, b1, W2, b2):
    BF = ml_dtypes.bfloat16
    F8N = ml_dtypes.float8_e4m3
    x = np.asarray(x, dtype=np.float32)
    A_qk = np.asarray(Wq, np.float32).T @ np.asarray(Wk, np.float32)
    shared = {
        "A_qk": np.ascontiguousarray(A_qk.astype(BF)),
        "WvT": np.ascontiguousarray(np.asarray(Wv, np.float32).T.astype(BF)),
        "W1T": np.ascontiguousarray(np.asarray(W1, np.float32).T.astype(BF)),
        "W2T": np.ascontiguousarray(np.asarray(W2, np.float32).T.astype(BF)),
        "b1": np.asarray(b1, np.float32),
        "b2": np.asarray(b2, np.float32),
        "ln_g": np.asarray(ln_g, np.float32),
        "ln_b": np.asarray(ln_b, np.float32),
        "ones128": np.ones(128, np.float32),
    }
    in_maps = []
    for c in range(N_CORES):
        b, h = divmod(c, 2)
        xT = x[:, b, :].T  # (512, 2048)
        q = xT[:, h * SQ:(h + 1) * SQ]
        o = xT[:, (1 - h) * SQ:(2 - h) * SQ]
        xp = np.concatenate([q, o], axis=1)  # (512, 2048), q-half first
        in_maps.append({
            "xT": np.ascontiguousarray(xp.astype(BF)),
            "xTM": np.ascontiguousarray(xp.T.astype(BF)),
            **shared,
        })
    return in_maps


def kernel(x, Wq, Wk, Wv, ln_g, ln_b, W1, b1, W2, b2):
    nc = _get_nc()
    in_maps = make_in_maps(x, Wq, Wk, Wv, ln_g, ln_b, W1, b1, W2, b2)
    res = run_bass_kernel_spmd(nc, in_maps, list(range(N_CORES)))
    out = np.empty((S, B, D), dtype=np.float32)
    for c in range(N_CORES):
        b, h = divmod(c, 2)
        out[h * SQ:(h + 1) * SQ, b, :] = res.results[c]["outT"].T.astype(np.float32)
    return out


# revision 65
# speedup vs baseline: 799.0649x; 1.0126x over previous
"""Trainium2 Bass kernel for the attention+LN+MLP block (nn_Attention_84310208020626).

Reference computation (per batch b):
    q = x_b @ Wq.T ; k = x_b @ Wk.T ; v = x_b @ Wv.T          (S=2048, D=512)
    attn = softmax(q k^T / sqrt(512))
    res  = attn @ v
    h    = LayerNorm(res) * ln_g + ln_b
    out  = relu(h @ W1.T + b1) @ W2.T + b2

Sharding: 8 cores = 4 batches x 2 sequence halves. Every core computes its
batch's full K/V (recompute, no collectives) and runs attention + LN + MLP
for its own 1024 query rows.

Device layout: activations are feature-major [feature, seq] so every GEMM
contracts over the partition dimension without transposes:
    GT[d',s]     = A-stationary GEMM over x, A = Wq^T Wk precomputed on host
                   (scores = q k^T = (x A) x^T, so no separate Q/K GEMMs)
    scoresT[t,s] = x-stationary GEMM, rhs = GT -> ACT exp -> bf16
    Z[d,s]       = xtm-stationary GEMM over exp (t-major x)
    resU[e,s]    = Wv-stationary GEMM over z (softmax denom NOT applied)
The softmax division is folded into LayerNorm via scale invariance (the
eps term is dropped: eps/var(attn out) ~ 4e-3, inside the accuracy budget),
and the LN mean/scale are folded into the MLP1 GEMM:
    P    = G1 @ res + r1 (x) (-mu)     (rank-1 K=1 matmul rides in the psum)
    h1   = relu( P * rstd[s] + (W1@ln_b)[f] + b1[f] )
with G1 = W1*diag(ln_g), r1 = G1 column sums (computed on device once).
rstd is broadcast across partitions with a K=1 ones matmul.

Scheduling: PE clock warm-up dummies; one activation-table load per table
set (exp set, then a dummy Sqrt pulls the sqrt-set load off the stats
chain); scores/z emission interleaved so the PE fills the ACT exp-stream
window; z GEMMs accumulate in a dedicated 2-bank psum pool; psum copies and
epilogue ops are spread across ACT/DVE/Pool respecting engine ISA limits
(Pool cannot touch PSUM, TensorScalarPtr is ACT/DVE-only); MLP2 accumulates
fc-major so it overlaps the h1 epilogue; outputs DMA per-chunk on SP/Pool.
"""

import ml_dtypes
import numpy as np

import concourse.bass as bass
import concourse.mybir as mybir
import concourse.tile as tile
from concourse import bacc
from concourse.bass_utils import run_bass_kernel_spmd

S, B, D = 2048, 4, 512
N_CORES = 8
SQ = 1024          # query rows per core
SBLK = 512         # s-block (pipeline granularity)
NBLK = SQ // SBLK  # 2
ND = D // 128      # 4 chunks of the feature dims
NDH = D // 256     # 2 double-row chunks of the feature dims
NT = S // 128      # 16 t-chunks
NTH = S // 256     # 8 double-row t-chunks
EPS = 1e-5
SCALE = 1.0 / float(np.sqrt(512.0))

F32 = mybir.dt.float32
F32R = mybir.dt.float32r
BF16 = mybir.dt.bfloat16
F8 = mybir.dt.float8e4
AF = mybir.ActivationFunctionType
ALU = mybir.AluOpType
DR = mybir.MatmulPerfMode.DoubleRow


def _emit(nc, tc, n_iters=1):
    xT = nc.tensor_by_name["xT"].ap()         # (512, 2048) bf16 d-major, q first
    xTM = nc.tensor_by_name["xTM"].ap()       # (2048, 512) bf16 t-major
    A_qk = nc.tensor_by_name["A_qk"].ap()     # (512, 512) = Wq.T @ Wk  (d, d')
    WvT = nc.tensor_by_name["WvT"].ap()       # (512, 512) bf16 = Wv.T (d, e)
    W1T = nc.tensor_by_name["W1T"].ap()       # (512, 512) = W1.T  (e, f)
    W2T = nc.tensor_by_name["W2T"].ap()
    b1 = nc.tensor_by_name["b1"].ap()         # (512,)
    b2 = nc.tensor_by_name["b2"].ap()
    ln_g = nc.tensor_by_name["ln_g"].ap()
    ln_b = nc.tensor_by_name["ln_b"].ap()
    outT = nc.tensor_by_name["outT"].ap()     # (512, 1024) bf16 out

    # ---------------- SBUF tiles ----------------
    from contextlib import ExitStack
    ctx = ExitStack()
    consts = ctx.enter_context(tc.tile_pool(name="consts", bufs=1))
    big = ctx.enter_context(tc.tile_pool(name="big", bufs=1))
    g8_pool = ctx.enter_context(tc.tile_pool(name="g8p", bufs=2))
    expb_pool = ctx.enter_context(tc.tile_pool(name="expbp", bufs=18))
    exp8_pool = ctx.enter_context(tc.tile_pool(name="exp8p", bufs=2))
    res_pool = ctx.enter_context(tc.tile_pool(name="resp", bufs=2))
    h1_pool = ctx.enter_context(tc.tile_pool(name="h1p", bufs=1))
    out_pool = ctx.enter_context(tc.tile_pool(name="outp", bufs=1))
    sq_pool = ctx.enter_context(tc.tile_pool(name="sqp", bufs=4))
    row_pool = ctx.enter_context(tc.tile_pool(name="rowp", bufs=1))
    bc_pool = ctx.enter_context(tc.tile_pool(name="bcp", bufs=2))

    mm_psum = ctx.enter_context(tc.tile_pool(name="mmps", bufs=6, space="PSUM"))
    z_psum = ctx.enter_context(tc.tile_pool(name="zps", bufs=2, space="PSUM"))

    # constants / weights (loaded once; Pool queue for the early ones, SP for
    # the late-needed MLP weights so the x DMAs keep their queues)
    a_sb = consts.tile([128, ND, D], BF16)    # (p, dc, d')
    wv_sb = consts.tile([128, ND, D], BF16)
    w1_sb = consts.tile([128, ND, D], BF16)
    w2_sb = consts.tile([128, ND, D], BF16)
    b1_sb = consts.tile([128, ND], F32)
    b2_sb = consts.tile([128, ND], F32)
    g_sb = consts.tile([128, ND], F32)
    lb_sb = consts.tile([128, ND], F32)
    ar = A_qk.rearrange("(dc p) e -> p dc e", p=128)
    nc.gpsimd.dma_start(out=a_sb[:, :, :], in_=ar[:, :, :])
    for v_sb, v_dram in ((b1_sb, b1), (b2_sb, b2), (g_sb, ln_g), (lb_sb, ln_b)):
        nc.gpsimd.dma_start(out=v_sb[:, :],
                            in_=v_dram.rearrange("(c p) -> p c", p=128))
    w1r_d = W1T.rearrange("(dc p) e -> p dc e", p=128)
    nc.gpsimd.dma_start(out=w1_sb[:, :, :], in_=w1r_d[:, :, :])
    wvr = WvT.rearrange("(dc p) e -> p dc e", p=128)
    nc.gpsimd.dma_start(out=wv_sb[:, :, :], in_=wvr[:, :, :])

    gb_sb = consts.tile([128, ND, 2], BF16)
    r1row_sb = consts.tile([1, D], F32R)
    rw_sb = consts.tile([128, ND, 2], F32)
    w1bb1_sb = consts.tile([128, ND], F32)

    nc.vector.tensor_copy(out=gb_sb[:, :, 0], in_=g_sb[:, :])
    nc.vector.tensor_copy(out=gb_sb[:, :, 1], in_=lb_sb[:, :])

    def emit_ln_fold_precompute_a():
        # r1[f] = sum_e W1[f,e] g[e],  w1b[f] = sum_e W1[f,e] ln_b[e]
        # then G1 = W1 * g[e] in place (folds LayerNorm into the MLP1 GEMM)
        for fc in range(ND):
            rps1 = mm_psum.tile([128, 2], F32, tag="mm")
            for ec in range(ND):
                nc.tensor.matmul(
                    rps1[:, :],
                    w1_sb[:, ec, fc * 128:(fc + 1) * 128],
                    gb_sb[:, ec, :],
                    start=(ec == 0), stop=(ec == ND - 1),
                )
            nc.vector.tensor_copy(out=rw_sb[:, fc, :], in_=rps1[:, :])
        nc.vector.tensor_add(out=w1bb1_sb[:, :], in0=rw_sb[:, :, 1], in1=b1_sb[:, :])
        for ec in range(ND):
            nc.vector.tensor_scalar_mul(out=w1_sb[:, ec, :], in0=w1_sb[:, ec, :],
                                        scalar1=g_sb[:, ec:ec + 1])

    def emit_ln_fold_precompute_b():
        # r1 as a [1, 512] row (K=1 stationary for the rank-1 -mu*r1 term
        # folded into the MLP1 GEMM); emitted after scores(0) so the PE
        # doesn't stall on the DVE g-scaling chain
        r1ps = mm_psum.tile([1, 512], F32, tag="mm")
        for ec in range(ND):
            nc.tensor.matmul(
                r1ps[:, :], ones_col_b[:, :], w1_sb[:, ec, :],
                start=(ec == 0), stop=(ec == ND - 1),
            )
        with nc.allow_low_precision(reason="float32r output is full fp32 width"):
            nc.scalar.copy(out=r1row_sb[:, :], in_=r1ps[:, :])

    ones128 = nc.tensor_by_name["ones128"].ap()  # (128,) of 1.0
    ones_col_b = consts.tile([128, 1], BF16)   # stationary for column sums
    nc.vector.memset(ones_col_b, 1.0)
    ones_row = consts.tile([1, 128], F32R)      # stationary for partition broadcast
    nc.gpsimd.dma_start(out=ones_row[:, :],
                        in_=ones128.bitcast(F32R).rearrange("(c p) -> c p", c=1))

    for _iter in range(n_iters):
        _emit_iter(nc, tc, xT, xTM, W1T, W2T, outT,
                   big, g8_pool, expb_pool, exp8_pool, res_pool, h1_pool,
                   out_pool, sq_pool, row_pool, bc_pool, mm_psum, z_psum,
                   a_sb, wv_sb, w1_sb, w2_sb,
                   b1_sb, b2_sb, g_sb, lb_sb,
                   ones_col_b, ones_row, rw_sb, w1bb1_sb, r1row_sb,
                   (emit_ln_fold_precompute_a, emit_ln_fold_precompute_b)
                   if _iter == 0 else None)

    ctx.close()


def _emit_iter(nc, tc, xT, xTM, W1T, W2T, outT,
               big, g8_pool, expb_pool, exp8_pool, res_pool, h1_pool,
               out_pool, sq_pool, row_pool, bc_pool, mm_psum, z_psum,
               a_sb, wv_sb, w1_sb, w2_sb,
               b1_sb, b2_sb, g_sb, lb_sb,
               ones_col_b, ones_row, rw_sb, w1bb1_sb, r1row_sb,
               precompute=None):
    # PE clock warm-up: dummy matmuls on a zeroed tile keep the PE busy
    # from ~0.4us so the p-state ramp completes before the real GEMMs
    warm_sb = big.tile([128, 256], BF16, tag="warm", name="warm_sb")
    nc.vector.memset(warm_sb, 0.0)
    wps = mm_psum.tile([1, 256], F32, tag="mm")
    for _ in range(11):
        nc.tensor.matmul(wps[:, :], ones_col_b[:, :], warm_sb[:, :],
                         start=True, stop=True)

    # x bf16 d-major (p, dc, t): scores stationary and GT moving operand,
    # loaded t-chunk-wise so GT can start early
    x_sb = big.tile([128, ND, S], BF16, tag="x", name="x_sb")
    xr = xT.rearrange("(dc p) t -> p dc t", p=128)
    nc.sync.dma_start(out=x_sb[:, :, 0:512], in_=xr[:, :, 0:512])
    nc.sync.dma_start(out=x_sb[:, :, 512:1024], in_=xr[:, :, 512:1024])
    nc.sync.dma_start(out=x_sb[:, :, 1024:2048], in_=xr[:, :, 1024:2048])
    # x bf16 t-major (p, tc, d): Z stationary
    xtm_sb = big.tile([128, NT, D], BF16, tag="xtm", name="xtm_sb")
    xmr = xTM.rearrange("(tc p) d -> p tc d", p=128)
    nc.sync.dma_start(out=xtm_sb[:, 0:8, :], in_=xmr[:, 0:8, :])
    nc.sync.dma_start(out=xtm_sb[:, 8:16, :], in_=xmr[:, 8:16, :])

    # x fp8 t-major with DoubleRow pairing (p, tch, i, d): Z stationary

    # W2 on the SP queue after the x tensors (needed late)
    if precompute is not None:
        w2r_d = W2T.rearrange("(dc p) e -> p dc e", p=128)
        nc.sync.dma_start(out=w2_sb[:, :, :], in_=w2r_d[:, :, :])

    # ------- GT = A-stationary GEMM (G = x @ A; scores = G @ x^T) -------
    # qt copies alternate ACT/DVE so scores can start promptly
    qt_tiles = []
    for sb in range(NBLK):
        s0 = sb * SBLK
        qt_sb = g8_pool.tile([128, ND, SBLK], BF16, tag="qt")
        for ec in range(ND):
            qps = mm_psum.tile([128, 512], F32, tag="mm")
            for dc in range(ND):
                nc.tensor.matmul(
                    qps[:, :],
                    a_sb[:, dc, ec * 128:(ec + 1) * 128],
                    x_sb[:, dc, s0:s0 + SBLK],
                    start=(dc == 0), stop=(dc == ND - 1),
                )
            if ec % 2 == 0:
                nc.scalar.copy(out=qt_sb[:, ec, :], in_=qps[:, :])
            else:
                nc.vector.tensor_copy(out=qt_sb[:, ec, :], in_=qps[:, :])
        qt_tiles.append(qt_sb)

    # ---------------- per s-block pipeline (software-pipelined emission) ----
    exp8_tiles = [None] * NBLK
    er8_tiles = [None] * NBLK
    res_tiles = [None] * NBLK
    sums_tiles = [None] * NBLK
    rows0_tiles = [None] * NBLK
    rows2_tiles = [None] * NBLK

    def emit_scores_half(sb, half):
        # scoresT[t, s] bf16: stationary x chunk, moving qt
        qt_sb = qt_tiles[sb]
        if half == 0:
            exp_sb = exp8_pool.tile([128, NT, SBLK], BF16, tag="exp",
                                    name=f"exp_{sb}")
            exp8_tiles[sb] = exp_sb
        exp_sb = exp8_tiles[sb]
        for tc_i in range(half * (NT // 2), (half + 1) * (NT // 2)):
            sps = mm_psum.tile([128, 512], F32, tag="mm")
            for dc in range(ND):
                nc.tensor.matmul(
                    sps[:, :],
                    x_sb[:, dc, tc_i * 128:(tc_i + 1) * 128],
                    qt_sb[:, dc, :],
                    start=(dc == 0), stop=(dc == ND - 1),
                )
            nc.scalar.activation(out=exp_sb[:, tc_i, :], in_=sps[:, :],
                                 func=AF.Exp, scale=SCALE)

    z_tiles = [None] * NBLK

    def emit_z_pair(sb, dcs):
        # Z[d, s] = sum_t x[t,d] * exp[t,s]  (bf16), two dc chunks at a
        # time in the dedicated z psum pool
        exp_sb = exp8_tiles[sb]
        if z_tiles[sb] is None:
            z_tiles[sb] = sq_pool.tile([128, ND, SBLK], BF16, tag="z",
                                       name=f"z{sb}")
        z_sb = z_tiles[sb]
        for dc in dcs:
            zps = z_psum.tile([128, 512], F32, tag="z")
            for tc_i in range(NT):
                nc.tensor.matmul(
                    zps[:, :],
                    xtm_sb[:, tc_i, dc * 128:(dc + 1) * 128],
                    exp_sb[:, tc_i, :],
                    start=(tc_i == 0), stop=(tc_i == NT - 1),
                )
            nc.vector.tensor_copy(out=z_sb[:, dc, :], in_=zps[:, :])

    def emit_sums_res(sb):
        exp_sb = exp8_tiles[sb]
        z_sb = z_tiles[sb]
        # resU[e, s] = Wv @ Z  (bf16); copies alternate ACT/DVE so the
        # stats GEMMs start sooner
        res_sb = res_pool.tile([128, ND, SBLK], BF16, tag="res", name=f"res{sb}")
        for ec in range(ND):
            rps = mm_psum.tile([128, 512], F32, tag="mm")
            for dc in range(ND):
                nc.tensor.matmul(
                    rps[:, :],
                    wv_sb[:, dc, ec * 128:(ec + 1) * 128],
                    z_sb[:, dc, :],
                    start=(dc == 0), stop=(dc == ND - 1),
                )
            if ec % 2 == 0:
                nc.scalar.copy(out=res_sb[:, ec, :], in_=rps[:, :])
            else:
                nc.vector.tensor_copy(out=res_sb[:, ec, :], in_=rps[:, :])
        res_tiles[sb] = res_sb

    def emit_stats(sb):
        res_sb = res_tiles[sb]
        sume_ps = mm_psum.tile([1, 512], F32, tag="mm")
        if sb == 0:
            # mid-kernel block: fold the ec-chunks on Pool first so the PE
            # pays one matmul instead of four (the Pool latency hides under
            # the z(1) GEMMs)
            racc = sq_pool.tile([128, SBLK], BF16, tag="racc")
            nc.gpsimd.tensor_add(out=racc[:, :], in0=res_sb[:, 0, :],
                                 in1=res_sb[:, 1, :])
            nc.gpsimd.tensor_add(out=racc[:, :], in0=racc[:, :],
                                 in1=res_sb[:, 2, :])
            nc.gpsimd.tensor_add(out=racc[:, :], in0=racc[:, :],
                                 in1=res_sb[:, 3, :])
            nc.tensor.matmul(sume_ps[:, :], ones_col_b[:, :], racc[:, :],
                             start=True, stop=True)
        else:
            for ec in range(ND):
                nc.tensor.matmul(
                    sume_ps[:, :], ones_col_b[:, :],
                    res_sb[:, ec, :],
                    start=(ec == 0), stop=(ec == ND - 1),
                )
        sumsq_ps = mm_psum.tile([1, 512], F32, tag="mm")
        if sb == 0:
            sqacc = sq_pool.tile([128, SBLK], BF16, tag="sqacc")
            nc.gpsimd.tensor_mul(out=sqacc[:, :], in0=res_sb[:, 0, :],
                                 in1=res_sb[:, 0, :])
            for ec in range(1, ND):
                sq_sb = sq_pool.tile([128, SBLK], BF16, tag="sq")
                nc.gpsimd.tensor_mul(out=sq_sb[:, :], in0=res_sb[:, ec, :],
                                     in1=res_sb[:, ec, :])
                nc.gpsimd.tensor_add(out=sqacc[:, :], in0=sqacc[:, :],
                                     in1=sq_sb[:, :])
            nc.tensor.matmul(sumsq_ps[:, :], ones_col_b[:, :], sqacc[:, :],
                             start=True, stop=True)
        else:
            for ec in range(ND):
                sq_sb = sq_pool.tile([128, SBLK], BF16, tag="sq")
                nc.gpsimd.tensor_mul(out=sq_sb[:, :], in0=res_sb[:, ec, :],
                                     in1=res_sb[:, ec, :])
                nc.tensor.matmul(
                    sumsq_ps[:, :], ones_col_b[:, :],
                    sq_sb[:, :],
                    start=(ec == 0), stop=(ec == ND - 1),
                )

        # row stats on one partition:
        #   muU = sumE/512 ; varU = sumSq/512 - muU^2
        #   rstd = (varU + EPS*sums^2)^-0.5 ; murstd = muU*rstd
        # ACT funcs here (scaled copy, Square) share the exp table set; the
        # power -0.5 runs on DVE so ACT never reloads tables.
        rows = row_pool.tile([1, 4, SBLK], F32, tag="rows", name=f"rows{sb}")
        rows0 = row_pool.tile([1, SBLK], F32R, tag="rows0", name=f"rows0{sb}")
        rows2 = row_pool.tile([1, 2, SBLK], F32R, tag="rows2", name=f"rows2{sb}")
        with nc.allow_low_precision(reason="float32r output is full fp32 width"):
            nc.scalar.mul(out=rows0[:, :], in_=sume_ps[:, :], mul=-1.0 / D)  # -muU
        nc.scalar.mul(out=rows[:, 1, :], in_=sumsq_ps[:, :], mul=1.0 / D)    # msq
        # the eps*sums^2 correction is dropped: eps/var(attn out) ~ 4e-3,
        # well inside the accuracy budget (verified against the reference)
        nc.gpsimd.tensor_mul(out=rows[:, 3, :], in0=rows0[:, :], in1=rows0[:, :])
        nc.gpsimd.tensor_sub(out=rows[:, 1, :], in0=rows[:, 1, :], in1=rows[:, 3, :])
        nc.scalar.activation(out=rows[:, 1, :], in_=rows[:, 1, :], func=AF.Sqrt)
        with nc.allow_low_precision(reason="float32r output is full fp32 width"):
            nc.vector.reciprocal(out=rows2[:, 0, :], in_=rows[:, 1, :])      # rstd
        rows0_tiles[sb] = rows0
        rows2_tiles[sb] = rows2

    p_tiles = [None] * NBLK
    h1_tiles = [None] * NBLK

    def emit_p(sb):
        # P = G1 @ res + r1 x (-mu)  (bf16 GEMM + K=1 rank-1 term)
        res_sb = res_tiles[sb]
        rows0 = rows0_tiles[sb]
        p_ps = []
        for fc in range(ND):
            hps = mm_psum.tile([128, 512], F32, tag="mm", name=f"p{sb}_{fc}")
            for ec in range(ND):
                nc.tensor.matmul(
                    hps[:, :],
                    w1_sb[:, ec, fc * 128:(fc + 1) * 128],
                    res_sb[:, ec, :],
                    start=(ec == 0), stop=False,
                )
            nc.tensor.matmul(
                hps[:, :], r1row_sb[:, fc * 128:(fc + 1) * 128],
                rows0[:, :], start=False, stop=True,
            )
            p_ps.append(hps)
        p_tiles[sb] = p_ps

    def emit_bc_epi(sb):
        rows2 = rows2_tiles[sb]
        p_ps = p_tiles[sb]

        # broadcast rstd across 128 partitions via K=1 matmul (the -mu*r1
        # term already rides in the P psum, so no murstd broadcast needed)
        bc_sb = bc_pool.tile([128, 2, SBLK], F32, tag="bc_sb")
        bc_ps = z_psum.tile([128, 512], F32, tag="z")
        nc.tensor.matmul(
            bc_ps[:, :], ones_row[:, :],
            rows2[:, 0, :], start=True, stop=True,
        )
        nc.scalar.copy(out=bc_sb[:, 0, :], in_=bc_ps[:, :])

        # fused MLP1 + LayerNorm epilogue:
        #   h1 = relu( (P + r1*(-mu))*rstd[s] + w1b[f] + b1[f] )
        h1_sb = h1_pool.tile([128, ND, SBLK], BF16, tag="h1", name=f"h1_{sb}")
        for fc in range(ND):
            t_sb = sq_pool.tile([128, SBLK], F32R, tag="sq")
            # the P psum read must be on DVE (Pool cannot access PSUM)
            nc.vector.tensor_mul(out=t_sb[:, :], in0=p_ps[fc][:, :],
                                 in1=bc_sb[:, 0, :])
            if fc % 2 == 0:
                nc.scalar.activation(out=h1_sb[:, fc, :], in_=t_sb[:, :],
                                     func=AF.Relu, bias=w1bb1_sb[:, fc:fc + 1])
            else:
                nc.vector.tensor_scalar(
                    out=h1_sb[:, fc, :], in0=t_sb[:, :],
                    scalar1=w1bb1_sb[:, fc:fc + 1], scalar2=0.0,
                    op0=ALU.add, op1=ALU.max,
                )
        h1_tiles[sb] = h1_sb

    def emit_mlp2(sb):
        # fc-major accumulation: the first 4 MMs need only h1[fc=0], so the
        # GEMM overlaps the Pool/DVE epilogue chain producing h1.
        s0 = sb * SBLK
        h1_sb = h1_tiles[sb]
        o_sb = out_pool.tile([128, ND, SBLK], BF16, tag="o")
        outr = outT[:, s0:s0 + SBLK].rearrange("(gc p) s -> p gc s", p=128)
        ops = [mm_psum.tile([128, 512], F32, tag="mm", name=f"o{sb}_{gc}")
               for gc in range(ND)]
        for fc in range(ND):
            for gc in range(ND):
                nc.tensor.matmul(
                    ops[gc][:, :],
                    w2_sb[:, fc, gc * 128:(gc + 1) * 128],
                    h1_sb[:, fc, :],
                    start=(fc == 0), stop=(fc == ND - 1),
                )
        dma_engs = (nc.sync, nc.gpsimd, nc.sync, nc.gpsimd)
        for gc in range(ND):
            if gc % 2 == 0:
                nc.scalar.activation(out=o_sb[:, gc, :], in_=ops[gc][:, :],
                                     func=AF.Identity, bias=b2_sb[:, gc:gc + 1])
            else:
                nc.vector.tensor_scalar_add(out=o_sb[:, gc, :],
                                            in0=ops[gc][:, :],
                                            scalar1=b2_sb[:, gc:gc + 1])
            dma_engs[gc].dma_start(out=outr[:, gc, :], in_=o_sb[:, gc, :])

    if precompute is not None:
        precompute[0]()
    emit_scores_half(0, 0)
    emit_scores_half(0, 1)
    if precompute is not None:
        precompute[1]()
    emit_scores_half(1, 0)
    emit_z_pair(0, (0, 1))
    emit_scores_half(1, 1)
    # dummy Sqrt right after the exps: pulls the one exp->sqrt activation
    # table reload off the stats critical chain (all later ACT funcs are in
    # the sqrt table set)
    dummy_in = row_pool.tile([1, 4], F32, tag="dummyi")
    dummy_sq = row_pool.tile([1, 4], F32, tag="dummy")
    nc.vector.memset(dummy_in, 1.0)
    nc.scalar.activation(out=dummy_sq[:, :], in_=dummy_in[:, :], func=AF.Sqrt)
    emit_z_pair(0, (2, 3))
    emit_sums_res(0)
    emit_stats(0)
    emit_z_pair(1, (0, 1))
    emit_z_pair(1, (2, 3))
    emit_sums_res(1)
    emit_stats(1)
    emit_p(0)
    emit_bc_epi(0)
    emit_p(1)
    emit_bc_epi(1)
    emit_mlp2(0)
    emit_mlp2(1)


def build_nc(n_iters=1):
    nc = bacc.Bacc("TRN2", target_bir_lowering=False, debug=False)
    nc.tensor_by_name = {}

    def dram(name, shape, kind, dt=F32):
        t = nc.dram_tensor(name, shape, dt, kind=kind)
        nc.tensor_by_name[name] = t
        return t

    dram("xT", [D, S], "ExternalInput", BF16)
    dram("xTM", [S, D], "ExternalInput", BF16)
    for nm in ("A_qk", "WvT", "W1T", "W2T"):
        dram(nm, [D, D], "ExternalInput", BF16)
    for nm in ("b1", "b2", "ln_g", "ln_b"):
        dram(nm, [D], "ExternalInput")
    dram("ones128", [128], "ExternalInput")
    dram("outT", [D, SQ], "ExternalOutput", BF16)

    with tile.TileContext(nc) as tc:
        _emit(nc, tc, n_iters=n_iters)
    nc.compile()
    return nc


_CACHED_NC = None


def _get_nc():
    global _CACHED_NC
    if _CACHED_NC is None:
        _CACHED_NC = build_nc()
    return _CACHED_NC


def make_in_maps(x, Wq, Wk, Wv, ln_g, ln_b, W1, b1, W2, b2):
    BF = ml_dtypes.bfloat16
    F8N = ml_dtypes.float8_e4m3
    x = np.asarray(x, dtype=np.float32)
    A_qk = np.asarray(Wq, np.float32).T @ np.asarray(Wk, np.float32)
    shared = {
        "A_qk": np.ascontiguousarray(A_qk.astype(BF)),
        "WvT": np.ascontiguousarray(np.asarray(Wv, np.float32).T.astype(BF)),
        "W1T": np.ascontiguousarray(np.asarray(W1, np.float32).T.astype(BF)),
        "W2T": np.ascontiguousarray(np.asarray(W2, np.float32).T.astype(BF)),
        "b1": np.asarray(b1, np.float32),
        "b2": np.asarray(b2, np.float32),
        "ln_g": np.asarray(ln_g, np.float32),
        "ln_b": np.asarray(ln_b, np.float32),
        "ones128": np.ones(128, np.float32),
    }
    in_maps = []
    for c in range(N_CORES):
        b, h = divmod(c, 2)
        xT = x[:, b, :].T  # (512, 2048)
        q = xT[:, h * SQ:(h + 1) * SQ]
        o = xT[:, (1 - h) * SQ:(2 - h) * SQ]
        xp = np.concatenate([q, o], axis=1)  # (512, 2048), q-half first
        in_maps.append({
            "xT": np.ascontiguousarray(xp.astype(BF)),
            "xTM": np.ascontiguousarray(xp.T.astype(BF)),
            **shared,
        })
    return in_maps


def kernel(x, Wq, Wk, Wv, ln_g, ln_b, W1, b1, W2, b2):
    nc = _get_nc()
    in_maps = make_in_maps(x, Wq, Wk, Wv, ln_g, ln_b, W1, b1, W2, b2)
    res = run_bass_kernel_spmd(nc, in_maps, list(range(N_CORES)))
    out = np.empty((S, B, D), dtype=np.float32)
    for c in range(N_CORES):
        b, h = divmod(c, 2)
        out[h * SQ:(h + 1) * SQ, b, :] = res.results[c]["outT"].T.astype(np.float32)
    return out
